# revision 24
# baseline (speedup 1.0000x reference)
"""Trainium2 Bass kernel for the CGC multi-task MoE routing module.

Math: everything folds into one skinny matmul z = x @ A + d with A: [I, 24]
(padded to 32), followed by a per-sample 6-way softmax-weighted average
(see _fold).

v9 design (per core, BS=8192 samples, 16 bands of 512):
  - x quantized to fp8 e3m4 on host (rel err ~1.0e-2 < 2e-2 budget), packed
    HBM-contiguous per load so every load is one contiguous read with one
    descriptor per partition.
  - Load schedule: singles for the ramp bands 0/1 and the tail bands 14/15
    (fine-grained availability), pairs for the middle bands 2..13 (half the
    descriptor-generation work).  5 loads per HWDGE ring (even->sync,
    odd->scalar) stays under the ring's outstanding-DMA admission limit, so
    the scalar ENGINE queue is free for prompt per-quad bias/exp work
    (admission-stalled gens otherwise push every epilogue into the
    post-stream tail).  A leads the sync ring; d4 rides gpsimd SWDGE.
  - Matmul: stationary A bf16, moving x fp8e3 (mixed dtypes multiply
    exactly in the PE fp22 path).  PE column-group tiling
    (tile_position=(0,32j)) packs a quad's 4 bands into ONE PSUM bank
    [128, 512] -> a single full-partition bias-add per quad.
  - Epilogue per quad: DVE 32x32 block transpose, exp on logit lanes,
    softmax-weighted average with reciprocal_approx_fast.
  - The last quad is split into two 256-sample column halves with separate
    PSUM tiles and chains so the post-stream serial tail is halved.
"""

import os

import numpy as np

B, I, H = 65536, 512, 128
T, ES, EC = 2, 2, 4
ETOT = ES + EC

N_CORES = 8
BS = B // N_CORES  # samples per core
M = 32  # folded output channels, padded 24 -> 32
GW = 512  # samples per band (one PSUM bank column span)
NBAND = BS // GW  # 16 bands
NCHUNK = I // 128
NQ = 4  # quads of 4 bands; one PSUM bank per quad
NB = GW // 32  # 32-sample blocks per band
NWARM = 44  # PE p-state warmup matmuls


def _fold(inputs):
    """Fold all weights into A [128, NCHUNK, M] (bf16) and bias d4 [128,1] f32.

    Channel layout per task t (base 12*t): 0:6 gate logits, 6:12 per-expert
    scalars (bt folded in, valid since softmax weights sum to 1).
    A is packed so that partition p, chunk c holds row c*128+p of the
    [I, M] matrix.  d4 is d tiled x4 across partitions to match the
    4-band col-group PSUM layout.
    """
    import ml_dtypes

    w64 = lambda k: np.asarray(inputs[k], np.float64)
    Wc, bc, Ws, bs = w64("Wc"), w64("bc"), w64("Ws"), w64("bs")
    Wg, bg, Wt, bt = w64("Wg"), w64("bg"), w64("Wt"), w64("bt")

    A = np.zeros((I, M))
    d = np.zeros(M)
    for t in range(T):
        W_all = np.concatenate(
            [Ws[t, e] for e in range(ES)] + [Wc[e] for e in range(EC)], axis=1
        )  # [I, ETOT*H]
        b_all = np.concatenate(
            [bs[t, e] for e in range(ES)] + [bc[e] for e in range(EC)]
        )  # [ETOT*H]
        A[:, 12 * t : 12 * t + 6] = W_all @ Wg[t]
        d[12 * t : 12 * t + 6] = b_all @ Wg[t] + bg[t]
        A[:, 12 * t + 6 : 12 * t + 12] = (
            W_all.reshape(I, ETOT, H) * Wt[t, :, 0][None, None, :]
        ).sum(-1)
        d[12 * t + 6 : 12 * t + 12] = (
            b_all.reshape(ETOT, H) * Wt[t, :, 0][None, :]
        ).sum(-1) + bt[t, 0]
    Apack = (
        A.reshape(NCHUNK, 128, M).transpose(1, 0, 2).astype(ml_dtypes.bfloat16)
    )  # [128, NCHUNK, M]
    d4 = np.tile(d.astype(np.float32), 4).reshape(128, 1)
    return np.ascontiguousarray(Apack), d4


def _build_program():
    import concourse.bacc as bacc
    import concourse.mybir as mybir
    from concourse.tile import TileContext

    f32 = mybir.dt.float32
    bf16 = mybir.dt.bfloat16
    f8 = mybir.dt.float8e3

    nc = bacc.Bacc("TRN2", target_bir_lowering=False, debug=False, num_devices=N_CORES)
    xp_ext = nc.declare_dram_parameter("xp", [NBAND, 128, NCHUNK, GW], f8, isOutput=False)
    A_ext = nc.declare_dram_parameter("A", [128, NCHUNK, M], bf16, isOutput=False)
    d4_ext = nc.declare_dram_parameter("d4", [128, 1], f32, isOutput=False)
    # out[q, p, blk, t]: sample s = q*4*GW + (p//32)*GW + 32*blk + p%32, task t
    out_ext = nc.declare_dram_parameter("out", [NQ, 128, NB, T], f32, isOutput=True)

    with TileContext(nc) as tc:
        with (
            tc.tile_pool(name="consts", bufs=1) as cpool,
            tc.tile_pool(name="xin", bufs=1) as xpool,
            tc.tile_pool(name="zt", bufs=2) as ztpool,
            tc.tile_pool(name="zq", bufs=2) as zqpool,
            tc.tile_pool(name="epi", bufs=4) as epool,
            tc.tile_pool(name="psum", bufs=3, space="PSUM") as ppool,
        ):
            # A leads the sync ring (needed by the first matmul); d4 rides
            # gpsimd SWDGE (idle queue, needed only at the first bias-add).
            A_sb = cpool.tile([128, NCHUNK, M], bf16)
            nc.sync.dma_start(out=A_sb[:], in_=A_ext[:, :, :])
            d4_sb = cpool.tile([128, 1], f32)
            nc.gpsimd.dma_start(out=d4_sb[:], in_=d4_ext[:, :])

            # PE p-state pre-warm: fill the preamble-to-first-data window.
            warm = cpool.tile([128, 128], bf16, name="warm")
            nc.gpsimd.memset(warm[:], 0)
            warm_ps = ppool.tile([M, 128], f32, name="warm_ps", tag="warm", bufs=1)
            for _ in range(NWARM):
                nc.tensor.matmul(
                    warm_ps[:, :], warm[:, 0:M], warm[:, :], start=True, stop=True
                )

            # x bands in consumption order, alternating HWDGE rings.  The
            # last two bands swap rings so arrival order matches the quad-3
            # consumption order [12, 13, 15, 14] (both late bands sit at
            # ring position 8, landing nearly together at stream end).
            RING = {b: (nc.sync if b % 2 == 0 else nc.scalar) for b in range(NBAND)}
            RING[15], RING[14] = nc.sync, nc.scalar
            ISSUE = list(range(14)) + [15, 14]
            xs = {}
            for b in ISSUE:
                xb = xpool.tile([128, NCHUNK, GW], f8, name=f"x_{b}", tag=f"x{b}")
                RING[b].dma_start(out=xb[:], in_=xp_ext[b, :, :, :])
                xs[b] = xb

            ActT = mybir.ActivationFunctionType
            AxX = mybir.AxisListType.X
            AluAdd = mybir.AluOpType.add

            def epilogue(idx, Z, nblk, res_t):
                """softmax-weighted average on Z [128, nblk*32] -> res_t
                [128, nblk, T]."""
                Zb = Z.rearrange("p (blk c) -> p blk c", c=32)
                zt4 = Zb[:, :, 0:24].rearrange("p blk (t c) -> p blk t c", c=12)
                lg = zt4[:, :, :, 0:6]
                sc = zt4[:, :, :, 6:12]
                both = zt4.rearrange("p blk t (g c) -> p blk t g c", c=6)
                sums = epool.tile([128, nblk, T, 2], f32, name=f"sums_{idx}", tag="sums")
                rinv = epool.tile([128, nblk, T], f32, name=f"rinv_{idx}", tag="rinv")
                nc.scalar.activation(lg, lg, ActT.Exp)
                nc.vector.tensor_mul(sc, sc, lg)  # sc slot = exp * s
                nc.vector.tensor_reduce(sums[:], both, axis=AxX, op=AluAdd)
                nc.vector.reciprocal_approx_fast(out=rinv[:], in_=sums[:, :, :, 0])
                nc.vector.tensor_mul(res_t, sums[:, :, :, 1], rinv[:])

            def quad_chain(idx, psrc, ncols, out_slice):
                """bias-add + transpose + epilogue + store for psrc
                [128, ncols]; out_slice is the out_ext destination."""
                nblk = ncols // 32
                zT = ztpool.tile([128, ncols], f32, name=f"zT_{idx}", tag="zT")
                nc.scalar.add(zT[:], psrc, d4_sb[:])
                Zq = zqpool.tile([128, ncols], f32, name=f"Z_{idx}", tag="Z")
                nc.vector.transpose(Zq[:], zT[:])
                res = epool.tile([128, nblk, T], f32, name=f"res_{idx}", tag="res")
                epilogue(idx, Zq[:], nblk, res[:])
                nc.sync.dma_start(out=out_slice, in_=res[:])

            for q in range(NQ - 1):
                psZ = ppool.tile([128, GW], f32, name=f"psZ_{q}", tag="psZ")
                # band-outer: band j's 4 chunk-matmuls start as soon as its
                # load lands; col group j -> psum partitions 32j:32j+32
                for j in range(4):
                    xb = xs[4 * q + j]
                    for c in range(NCHUNK):
                        nc.tensor.matmul(
                            psZ[32 * j : 32 * j + 32, :],
                            A_sb[:, c, :],
                            xb[:, c, :],
                            start=(c == 0),
                            stop=(c == NCHUNK - 1),
                            tile_position=(0, 32 * j),
                        )
                quad_chain(q, psZ[:], GW, out_ext[q, :, :, :])

            # last quad: two 256-sample column halves with separate PSUM
            # tiles and chains, halving the post-stream serial tail.
            # Col-group band order [12, 13, 15, 14] matches arrival order.
            BANDQ3 = [12, 13, 15, 14]
            HW_ = GW // 2
            ps3 = [
                ppool.tile([128, HW_], f32, name=f"ps3_{h}", tag=f"ps3{h}", bufs=1)
                for h in range(2)
            ]
            for j in range(4):
                xb = xs[BANDQ3[j]]
                for h in range(2):
                    for c in range(NCHUNK):
                        nc.tensor.matmul(
                            ps3[h][32 * j : 32 * j + 32, :],
                            A_sb[:, c, :],
                            xb[:, c, h * HW_ : (h + 1) * HW_],
                            start=(c == 0),
                            stop=(c == NCHUNK - 1),
                            tile_position=(0, 32 * j),
                        )
            nh = NB // 2
            for h in range(2):
                quad_chain(
                    f"3_{h}", ps3[h][:], HW_, out_ext[3, :, h * nh : (h + 1) * nh, :]
                )

    nc.compile()
    return nc


_PROGRAM = None


def _ensure_ntff_hook():
    """Provide antenv.axon_hooks if the image lacks it (NTFF profiling)."""
    try:
        import antenv.axon_hooks  # noqa: F401

        return
    except ImportError:
        pass
    import contextlib
    import ctypes
    import sys
    import types

    import antenv

    mod = types.ModuleType("antenv.axon_hooks")
    holder = {"hook": None}
    mod.set_axon_ntff_profile_hook = lambda h: holder.__setitem__("hook", h)
    mod.get_axon_ntff_profile_hook = lambda: holder["hook"]
    sys.modules["antenv.axon_hooks"] = mod
    antenv.axon_hooks = mod

    so_path = "/opt/axon/libaxon_pjrt.so"
    try:
        lib = ctypes.CDLL(so_path)
    except OSError:
        return
    if not hasattr(lib, "axon_start_nrt_profile"):
        return
    lib.axon_start_nrt_profile.argtypes = [
        ctypes.POINTER(ctypes.c_int64),
        ctypes.c_size_t,
    ]
    lib.axon_start_nrt_profile.restype = ctypes.c_int64
    lib.axon_stop_nrt_profile.argtypes = [ctypes.c_char_p]
    lib.axon_stop_nrt_profile.restype = ctypes.c_int64

    @contextlib.contextmanager
    def _hook(output_dir, device_ids):
        import jax

        jax.devices()
        if device_ids:
            ids = (ctypes.c_int64 * len(device_ids))(*device_ids)
            rc = lib.axon_start_nrt_profile(ids, len(device_ids))
        else:
            rc = lib.axon_start_nrt_profile(None, 0)
        if rc != 0:
            raise RuntimeError(f"axon_start_nrt_profile rc={rc}")
        try:
            yield
        finally:
            n = lib.axon_stop_nrt_profile(str(output_dir).encode())
            print(f"ntff profile: {n} file(s) written to {output_dir}")

    mod.set_axon_ntff_profile_hook(_hook)


def _run(inputs, trace=False):
    global _PROGRAM
    import ml_dtypes

    import concourse.bass_utils as bass_utils

    if trace:
        _ensure_ntff_hook()
        bass_utils.upload_artifacts = lambda tmpdir: "local://" + tmpdir

    A, d4 = _fold(inputs)
    x8 = np.asarray(inputs["x"], np.float32).astype(ml_dtypes.float8_e3m4)
    in_maps = []
    for i in range(N_CORES):
        shard = x8[i * BS : (i + 1) * BS]  # [BS, I] fp8
        # xp[b, p, c, s] = x[b*GW + s, c*128 + p]
        xp = np.ascontiguousarray(
            shard.T.reshape(NCHUNK, 128, NBAND, GW).transpose(2, 1, 0, 3)
        )
        in_maps.append({"xp": xp, "A": A, "d4": d4})

    if _PROGRAM is None:
        _PROGRAM = _build_program()

    kres = bass_utils.run_bass_kernel_spmd(
        _PROGRAM, in_maps, core_ids=list(range(N_CORES)), trace=trace
    )

    parts = []
    for i in range(N_CORES):
        o = np.asarray(kres.results[i]["out"])  # [NQ, 128, NB, T]
        # s = q*4*GW + j*GW + 32*blk + r with p = 32*j + r; quad 3's col
        # groups hold bands [12, 13, 15, 14] -> permute its j axis
        o = o.reshape(NQ, 4, 32, NB, T).copy()  # q, j, r, blk, t
        o[3] = o[3][[0, 1, 3, 2]]
        parts.append(o.transpose(4, 0, 1, 3, 2).reshape(T, BS))
    full = np.concatenate(parts, axis=1)[:, :, None].astype(np.float32)
    return full, kres


def kernel(**inputs):
    out, _ = _run(inputs, trace=bool(int(os.environ.get("KERNEL_TRACE", "0"))))
    return out


# revision 25
# speedup vs baseline: 1.1285x; 1.1285x over previous
"""Trainium2 Bass kernel for the CGC multi-task MoE routing module.

Math: everything folds into one skinny matmul z = x @ A + d with A: [I, 24]
(padded to 32), followed by a per-sample 6-way softmax-weighted average
(see _fold).

Final design (per core, BS=8192 samples, 16 bands of 512):
  - x quantized to fp8 e3m4 on host (rel err ~1.0e-2 < 2e-2 budget), packed
    HBM-contiguous per 512-sample band [16, 128, 4, 512] so every load is
    one 256KB contiguous read with one 2KB descriptor per partition.
  - Band loads issued in consumption order, alternating the two HWDGE
    rings (even->sync, odd->scalar; a single ring's ~650ns/load descriptor
    generation would pace the stream at ~290 GB/s vs ~380 line rate).  The
    last two bands swap rings and quad 3 consumes bands [12, 13, 15, 14]
    so consumption order matches arrival order at stream end.  A leads the
    sync ring; d4 rides gpsimd SWDGE.
  - Matmul: stationary A bf16, moving x fp8e3 (mixed dtypes multiply
    exactly in the PE fp22 path; fp8 at single rate = bf16 speed, and the
    PE floor of 64 x 512 moving columns is the kernel's compute wall).
    PE column-group tiling (tile_position=(0,32j)) packs a quad's 4 bands
    into ONE PSUM bank [128, 512] -> a single full-partition bias-add per
    quad instead of four quarter-partition ones.
  - Epilogue per quad: scalar bias-add from PSUM, DVE 32x32 block
    transpose, exp on logit lanes, softmax-weighted average with
    reciprocal_approx_fast (1 DVE op vs 6-cycle exact reciprocal).
  - The last quad is split into two 256-sample column halves with separate
    PSUM tiles and chains so the post-stream serial tail is halved.
  - 44 warmup matmuls fill the ~7us framework preamble and hold the PE
    p-state up before the first real matmul.
"""

import os

import numpy as np

B, I, H = 65536, 512, 128
T, ES, EC = 2, 2, 4
ETOT = ES + EC

N_CORES = 8
BS = B // N_CORES  # samples per core
M = 32  # folded output channels, padded 24 -> 32
GW = 512  # samples per band (one PSUM bank column span)
NBAND = BS // GW  # 16 bands
NCHUNK = I // 128
NQ = 4  # quads of 4 bands; one PSUM bank per quad
NB = GW // 32  # 32-sample blocks per band
NWARM = 44  # PE p-state warmup matmuls


def _fold(inputs):
    """Fold all weights into A [128, NCHUNK, M] (bf16) and bias d4 [128,1] f32.

    Channel layout per task t (base 12*t): 0:6 gate logits, 6:12 per-expert
    scalars (bt folded in, valid since softmax weights sum to 1).
    A is packed so that partition p, chunk c holds row c*128+p of the
    [I, M] matrix.  d4 is d tiled x4 across partitions to match the
    4-band col-group PSUM layout.
    """
    import ml_dtypes

    w64 = lambda k: np.asarray(inputs[k], np.float64)
    Wc, bc, Ws, bs = w64("Wc"), w64("bc"), w64("Ws"), w64("bs")
    Wg, bg, Wt, bt = w64("Wg"), w64("bg"), w64("Wt"), w64("bt")

    A = np.zeros((I, M))
    d = np.zeros(M)
    for t in range(T):
        W_all = np.concatenate(
            [Ws[t, e] for e in range(ES)] + [Wc[e] for e in range(EC)], axis=1
        )  # [I, ETOT*H]
        b_all = np.concatenate(
            [bs[t, e] for e in range(ES)] + [bc[e] for e in range(EC)]
        )  # [ETOT*H]
        A[:, 12 * t : 12 * t + 6] = W_all @ Wg[t]
        d[12 * t : 12 * t + 6] = b_all @ Wg[t] + bg[t]
        A[:, 12 * t + 6 : 12 * t + 12] = (
            W_all.reshape(I, ETOT, H) * Wt[t, :, 0][None, None, :]
        ).sum(-1)
        d[12 * t + 6 : 12 * t + 12] = (
            b_all.reshape(ETOT, H) * Wt[t, :, 0][None, :]
        ).sum(-1) + bt[t, 0]
    Apack = (
        A.reshape(NCHUNK, 128, M).transpose(1, 0, 2).astype(ml_dtypes.bfloat16)
    )  # [128, NCHUNK, M]
    d4 = np.tile(d.astype(np.float32), 4).reshape(128, 1)
    return np.ascontiguousarray(Apack), d4


def _build_program():
    import concourse.bacc as bacc
    import concourse.mybir as mybir
    from concourse.tile import TileContext

    f32 = mybir.dt.float32
    bf16 = mybir.dt.bfloat16
    f8 = mybir.dt.float8e3

    nc = bacc.Bacc("TRN2", target_bir_lowering=False, debug=False, num_devices=N_CORES)
    xp_ext = nc.declare_dram_parameter("xp", [NBAND, 128, NCHUNK, GW], f8, isOutput=False)
    A_ext = nc.declare_dram_parameter("A", [128, NCHUNK, M], bf16, isOutput=False)
    d4_ext = nc.declare_dram_parameter("d4", [128, 1], f32, isOutput=False)
    # out[q, p, blk, t]: sample s = q*4*GW + (p//32)*GW + 32*blk + p%32, task t
    out_ext = nc.declare_dram_parameter("out", [NQ, 128, NB, T], f32, isOutput=True)

    with TileContext(nc) as tc:
        with (
            tc.tile_pool(name="consts", bufs=1) as cpool,
            tc.tile_pool(name="xin", bufs=1) as xpool,
            tc.tile_pool(name="zt", bufs=2) as ztpool,
            tc.tile_pool(name="zq", bufs=2) as zqpool,
            tc.tile_pool(name="epi", bufs=4) as epool,
            tc.tile_pool(name="psum", bufs=3, space="PSUM") as ppool,
        ):
            # A leads the sync ring (needed by the first matmul); d4 rides
            # gpsimd SWDGE (idle queue, needed only at the first bias-add).
            A_sb = cpool.tile([128, NCHUNK, M], bf16)
            nc.sync.dma_start(out=A_sb[:], in_=A_ext[:, :, :])
            d4_sb = cpool.tile([128, 1], f32)
            nc.gpsimd.dma_start(out=d4_sb[:], in_=d4_ext[:, :])

            # PE p-state pre-warm: fill the preamble-to-first-data window.
            warm = cpool.tile([128, 128], bf16, name="warm")
            nc.gpsimd.memset(warm[:], 0)
            warm_ps = ppool.tile([M, 128], f32, name="warm_ps", tag="warm", bufs=1)
            for _ in range(NWARM):
                nc.tensor.matmul(
                    warm_ps[:, :], warm[:, 0:M], warm[:, :], start=True, stop=True
                )

            # x bands in consumption order, alternating HWDGE rings.  The
            # last two bands swap rings so arrival order matches the quad-3
            # consumption order [12, 13, 15, 14] (both late bands sit at
            # ring position 8, landing nearly together at stream end).
            RING = {b: (nc.sync if b % 2 == 0 else nc.scalar) for b in range(NBAND)}
            RING[15], RING[14] = nc.sync, nc.scalar
            ISSUE = list(range(14)) + [15, 14]
            xs = {}
            for b in ISSUE:
                xb = xpool.tile([128, NCHUNK, GW], f8, name=f"x_{b}", tag=f"x{b}")
                RING[b].dma_start(out=xb[:], in_=xp_ext[b, :, :, :])
                xs[b] = xb

            ActT = mybir.ActivationFunctionType
            AxX = mybir.AxisListType.X
            AluAdd = mybir.AluOpType.add

            def epilogue(idx, Z, nblk, res_t):
                """softmax-weighted average on Z [128, nblk*32] -> res_t
                [128, nblk, T]."""
                Zb = Z.rearrange("p (blk c) -> p blk c", c=32)
                zt4 = Zb[:, :, 0:24].rearrange("p blk (t c) -> p blk t c", c=12)
                lg = zt4[:, :, :, 0:6]
                sc = zt4[:, :, :, 6:12]
                both = zt4.rearrange("p blk t (g c) -> p blk t g c", c=6)
                sums = epool.tile([128, nblk, T, 2], f32, name=f"sums_{idx}", tag="sums")
                rinv = epool.tile([128, nblk, T], f32, name=f"rinv_{idx}", tag="rinv")
                nc.scalar.activation(lg, lg, ActT.Exp)
                nc.vector.tensor_mul(sc, sc, lg)  # sc slot = exp * s
                nc.vector.tensor_reduce(sums[:], both, axis=AxX, op=AluAdd)
                nc.vector.reciprocal_approx_fast(out=rinv[:], in_=sums[:, :, :, 0])
                nc.vector.tensor_mul(res_t, sums[:, :, :, 1], rinv[:])

            def quad_chain(idx, psrc, ncols, out_slice):
                """bias-add + transpose + epilogue + store for psrc
                [128, ncols]; out_slice is the out_ext destination."""
                nblk = ncols // 32
                zT = ztpool.tile([128, ncols], f32, name=f"zT_{idx}", tag="zT")
                nc.scalar.add(zT[:], psrc, d4_sb[:])
                Zq = zqpool.tile([128, ncols], f32, name=f"Z_{idx}", tag="Z")
                nc.vector.transpose(Zq[:], zT[:])
                res = epool.tile([128, nblk, T], f32, name=f"res_{idx}", tag="res")
                epilogue(idx, Zq[:], nblk, res[:])
                nc.sync.dma_start(out=out_slice, in_=res[:])

            for q in range(NQ - 1):
                psZ = ppool.tile([128, GW], f32, name=f"psZ_{q}", tag="psZ")
                # band-outer: band j's 4 chunk-matmuls start as soon as its
                # load lands; col group j -> psum partitions 32j:32j+32
                for j in range(4):
                    xb = xs[4 * q + j]
                    for c in range(NCHUNK):
                        nc.tensor.matmul(
                            psZ[32 * j : 32 * j + 32, :],
                            A_sb[:, c, :],
                            xb[:, c, :],
                            start=(c == 0),
                            stop=(c == NCHUNK - 1),
                            tile_position=(0, 32 * j),
                        )
                quad_chain(q, psZ[:], GW, out_ext[q, :, :, :])

            # last quad: two 256-sample column halves with separate PSUM
            # tiles and chains, halving the post-stream serial tail.
            # Col-group band order [12, 13, 15, 14] matches arrival order.
            BANDQ3 = [12, 13, 15, 14]
            HW_ = GW // 2
            ps3 = [
                ppool.tile([128, HW_], f32, name=f"ps3_{h}", tag=f"ps3{h}", bufs=1)
                for h in range(2)
            ]
            for j in range(4):
                xb = xs[BANDQ3[j]]
                for h in range(2):
                    for c in range(NCHUNK):
                        nc.tensor.matmul(
                            ps3[h][32 * j : 32 * j + 32, :],
                            A_sb[:, c, :],
                            xb[:, c, h * HW_ : (h + 1) * HW_],
                            start=(c == 0),
                            stop=(c == NCHUNK - 1),
                            tile_position=(0, 32 * j),
                        )
            nh = NB // 2
            for h in range(2):
                quad_chain(
                    f"3_{h}", ps3[h][:], HW_, out_ext[3, :, h * nh : (h + 1) * nh, :]
                )

    nc.compile()
    return nc


_PROGRAM = None


def _ensure_ntff_hook():
    """Provide antenv.axon_hooks if the image lacks it (NTFF profiling)."""
    try:
        import antenv.axon_hooks  # noqa: F401

        return
    except ImportError:
        pass
    import contextlib
    import ctypes
    import sys
    import types

    import antenv

    mod = types.ModuleType("antenv.axon_hooks")
    holder = {"hook": None}
    mod.set_axon_ntff_profile_hook = lambda h: holder.__setitem__("hook", h)
    mod.get_axon_ntff_profile_hook = lambda: holder["hook"]
    sys.modules["antenv.axon_hooks"] = mod
    antenv.axon_hooks = mod

    so_path = "/opt/axon/libaxon_pjrt.so"
    try:
        lib = ctypes.CDLL(so_path)
    except OSError:
        return
    if not hasattr(lib, "axon_start_nrt_profile"):
        return
    lib.axon_start_nrt_profile.argtypes = [
        ctypes.POINTER(ctypes.c_int64),
        ctypes.c_size_t,
    ]
    lib.axon_start_nrt_profile.restype = ctypes.c_int64
    lib.axon_stop_nrt_profile.argtypes = [ctypes.c_char_p]
    lib.axon_stop_nrt_profile.restype = ctypes.c_int64

    @contextlib.contextmanager
    def _hook(output_dir, device_ids):
        import jax

        jax.devices()
        if device_ids:
            ids = (ctypes.c_int64 * len(device_ids))(*device_ids)
            rc = lib.axon_start_nrt_profile(ids, len(device_ids))
        else:
            rc = lib.axon_start_nrt_profile(None, 0)
        if rc != 0:
            raise RuntimeError(f"axon_start_nrt_profile rc={rc}")
        try:
            yield
        finally:
            n = lib.axon_stop_nrt_profile(str(output_dir).encode())
            print(f"ntff profile: {n} file(s) written to {output_dir}")

    mod.set_axon_ntff_profile_hook(_hook)


def _run(inputs, trace=False):
    global _PROGRAM
    import ml_dtypes

    import concourse.bass_utils as bass_utils

    if trace:
        _ensure_ntff_hook()
        bass_utils.upload_artifacts = lambda tmpdir: "local://" + tmpdir

    A, d4 = _fold(inputs)
    x8 = np.asarray(inputs["x"], np.float32).astype(ml_dtypes.float8_e3m4)
    in_maps = []
    for i in range(N_CORES):
        shard = x8[i * BS : (i + 1) * BS]  # [BS, I] fp8
        # xp[b, p, c, s] = x[b*GW + s, c*128 + p]
        xp = np.ascontiguousarray(
            shard.T.reshape(NCHUNK, 128, NBAND, GW).transpose(2, 1, 0, 3)
        )
        in_maps.append({"xp": xp, "A": A, "d4": d4})

    if _PROGRAM is None:
        _PROGRAM = _build_program()

    kres = bass_utils.run_bass_kernel_spmd(
        _PROGRAM, in_maps, core_ids=list(range(N_CORES)), trace=trace
    )

    parts = []
    for i in range(N_CORES):
        o = np.asarray(kres.results[i]["out"])  # [NQ, 128, NB, T]
        # s = q*4*GW + j*GW + 32*blk + r with p = 32*j + r; quad 3's col
        # groups hold bands [12, 13, 15, 14] -> permute its j axis
        o = o.reshape(NQ, 4, 32, NB, T).copy()  # q, j, r, blk, t
        o[3] = o[3][[0, 1, 3, 2]]
        parts.append(o.transpose(4, 0, 1, 3, 2).reshape(T, BS))
    full = np.concatenate(parts, axis=1)[:, :, None].astype(np.float32)
    return full, kres


def kernel(**inputs):
    out, _ = _run(inputs, trace=bool(int(os.environ.get("KERNEL_TRACE", "0"))))
    return out
